# revision 1
# baseline (speedup 1.0000x reference)
"""Multi-head attention (B=2, S=2048, D=1024, H=16, causal) on 8 Trainium2
NeuronCores via Bass/Tile.

Sharding: core c -> batch c//4, heads [4*(c%4), 4*(c%4)+4)  (DP over batch x
TP over heads).  QKV weights column-parallel, O row-parallel; the 4 partial
[S, D] outputs per batch are summed on the host (gather step), bias bo added
there too.

Per-core dataflow (bf16 matmuls, fp32 PSUM accumulation):
  - host supplies x.T [D, S] per batch (so the d_in contraction dim lands on
    SBUF partitions), plus pre-swizzled weight blocks.
  - Q/K proj -> qT/kT [dk_c=256, S] (head-major, 2 chunks of 128 = 2 heads).
  - V proj  -> natural [S, 260] layout: per head 64 cols of V plus a ones
    column (written via the bias row) for the flash-style softmax denominator.
  - scores computed transposed: sT[k, q] = kT.T @ qT per head; exp on ScalarE
    reads PSUM directly; causal mask applied only on diagonal blocks via
    precomputed 0/1 bf16 tiles; strictly-upper blocks skipped.
  - attn@V: outT[65, q] += V'[k,65].T @ expT[k,q]; row 64 accumulates the
    softmax denominator.  Normalize = reciprocal on lane 64 + PE outer-product
    broadcast + DVE multiply.
  - O proj: out[q, D] = sum_h outT_h[dk,q].T @ Wo_h[dk, D]; PSUM -> DRAM.
"""

import os
import sys
import types

import numpy as np

B, S, D, H = 2, 2048, 1024, 16
DK = D // H  # 64
N_CORES = 8
HPC = 4  # heads per core
SCALE = 1.0 / np.sqrt(np.float32(DK))  # folded into Wq/bq on host

QC = 512  # query block (free dim of scores matmuls)
NQC = S // QC  # 4
KC = 128  # key block (partition dim of transposed scores)
GK = 2  # key blocks per exp group -> scores psum tile [128, GK*QC]


def _install_ntff_hook():
    """The image's antenv lacks axon_hooks; register the NTFF profile hook
    ourselves so run_bass_kernel_spmd(trace=True) works."""
    if "antenv.axon_hooks" in sys.modules:
        return
    try:
        mod = types.ModuleType("antenv.axon_hooks")
        state = {"hook": None}
        mod.set_axon_ntff_profile_hook = lambda h: state.__setitem__("hook", h)
        mod.get_axon_ntff_profile_hook = lambda: state["hook"]
        sys.modules["antenv.axon_hooks"] = mod
        from trn_agent_boot.trn_boot import _ntff_profile_via_ctypes

        mod.set_axon_ntff_profile_hook(
            _ntff_profile_via_ctypes("/opt/axon/libaxon_pjrt.so")
        )
    except Exception:
        sys.modules.pop("antenv.axon_hooks", None)


def _split_multi_waits(nc):
    """This walrus build accepts at most ONE sem wait per instruction; Tile
    packs several.  Split extras into preceding single-wait NOPs on the same
    engine (equivalent semantics: the engine blocks on them in order)."""
    import bass_rust

    cnt = 0
    for bbw in nc.main_func.blocks:
        bb = bbw.bb if hasattr(bbw, "bb") else bbw
        out = []
        changed = False
        for ins in bb.instructions:
            si = ins.sync_info
            if si is not None and len(si.on_wait) > 1:
                changed = True
                waits = list(si.on_wait)
                for w in waits[:-1]:
                    cnt += 1
                    nop = bass_rust.InstNoOp(name=f"I-wsp{cnt}", ins=[], outs=[])
                    nop.engine = ins.engine
                    nop.sync_info = bass_rust.SyncInfo(on_wait=[w], on_update=[])
                    out.append(nop)
                si.on_wait = [waits[-1]]
                ins.sync_info = si
            out.append(ins)
        if changed:
            bb.instructions = out
    return cnt


def _build_nc(split=True, phase=5):
    from contextlib import ExitStack

    import concourse.bass as bass
    import concourse.tile as tile
    from concourse import mybir

    bf16 = mybir.dt.bfloat16
    f32 = mybir.dt.float32

    nc = bass.Bass()
    xqT = nc.declare_dram_parameter("xqT", [D, S], bf16, isOutput=False)
    xkT = nc.declare_dram_parameter("xkT", [D, S], bf16, isOutput=False)
    xvT = nc.declare_dram_parameter("xvT", [D, S], bf16, isOutput=False)
    wq = nc.declare_dram_parameter("wq", [128, 8 * 256], bf16, isOutput=False)
    wk = nc.declare_dram_parameter("wk", [128, 8 * 256], bf16, isOutput=False)
    wv = nc.declare_dram_parameter("wv", [128, 8 * 260], bf16, isOutput=False)
    wo = nc.declare_dram_parameter("wo", [64, 4 * 1024], bf16, isOutput=False)
    bq = nc.declare_dram_parameter("bq", [128, 2], f32, isOutput=False)
    bk = nc.declare_dram_parameter("bk", [128, 2], f32, isOutput=False)
    bvp = nc.declare_dram_parameter("bvp", [1, 260], f32, isOutput=False)
    cmask = nc.declare_dram_parameter("cmask", [128, 4 * QC], bf16, isOutput=False)
    outp = nc.declare_dram_parameter("outp", [S, D], f32, isOutput=True)

    with tile.TileContext(nc) as tc, ExitStack() as ctx:
        consts = ctx.enter_context(tc.tile_pool(name="consts", bufs=1))
        xs = ctx.enter_context(tc.tile_pool(name="xs", bufs=10))
        acts = ctx.enter_context(tc.tile_pool(name="acts", bufs=1))
        exps = ctx.enter_context(tc.tile_pool(name="exps", bufs=6))
        rcps = ctx.enter_context(tc.tile_pool(name="rcps", bufs=4))
        osb = ctx.enter_context(tc.tile_pool(name="osb", bufs=4))
        ps_small = ctx.enter_context(
            tc.tile_pool(name="ps_small", bufs=2, space="PSUM")
        )
        ps_sc = ctx.enter_context(tc.tile_pool(name="ps_sc", bufs=2, space="PSUM"))
        ps_av = ctx.enter_context(tc.tile_pool(name="ps_av", bufs=2, space="PSUM"))

        # ---- constants ----
        # wk + bk first: the K projection consumes them before anything else,
        # so the PE can start ~4us in instead of waiting on all constants.
        wk_sb = consts.tile([128, 8 * 256], bf16)
        nc.sync.dma_start(out=wk_sb[:], in_=wk[:])
        bk_sb = consts.tile([128, 2], f32)
        nc.sync.dma_start(out=bk_sb[:], in_=bk[:])
        wq_sb = consts.tile([128, 8 * 256], bf16, name="wq_sb")
        nc.sync.dma_start(out=wq_sb[:], in_=wq[:])
        bq_sb = consts.tile([128, 2], f32, name="bq_sb")
        nc.sync.dma_start(out=bq_sb[:], in_=bq[:])
        wv_sb = consts.tile([128, 8 * 260], bf16, name="wv_sb")
        nc.sync.dma_start(out=wv_sb[:], in_=wv[:])
        wo_sb = consts.tile([64, 4 * 1024], bf16, name="wo_sb")
        nc.sync.dma_start(out=wo_sb[:], in_=wo[:])
        bvp_sb = consts.tile([128, 260], f32, name="bvp_sb")
        nc.sync.dma_start(out=bvp_sb[:], in_=bvp[:].to_broadcast((128, 260)))
        cm_sb = consts.tile([128, 4 * QC], bf16, name="cm_sb")
        nc.sync.dma_start(out=cm_sb[:], in_=cmask[:])
        ones_sb = consts.tile([65, 64], bf16)
        nc.vector.memset(ones_sb[:], 1.0)

        # ---- projections ----
        # K then Q (qT/kT [dk-chunk 128 = 2 heads, S]), then V (natural).
        qt = [acts.tile([128, S], bf16, name=f"qt{m}", tag=f"qt{m}") for m in range(2)]
        kt = [acts.tile([128, S], bf16, name=f"kt{m}", tag=f"kt{m}") for m in range(2)]
        vh_sb = acts.tile([128, 16, 260], bf16, name="vh", tag="vh")
        outT = [
            acts.tile([64, S], bf16, name=f"outT{h}", tag=f"outT{h}")
            for h in range(HPC)
        ]

        def emit_kq_proj(src_, wsb, bsb, dst):
            if True:
                xt = {}
                for half in range(2):
                    for dc in range(8):
                        t = xs.tile([128, S // 2], bf16, name="xt", tag="xt")
                        nc.sync.dma_start(
                            out=t[:],
                            in_=src_[
                                dc * 128:(dc + 1) * 128,
                                half * 1024:(half + 1) * 1024,
                            ],
                        )
                        xt[(dc, half)] = t
                for half in range(2):
                    for m in range(2):
                        for scq in range(2):
                            sc = half * 2 + scq
                            ps = ps_small.tile([128, 512], f32, name="ps", tag="ps")
                            for dc in range(8):
                                nc.tensor.matmul(
                                    ps[:],
                                    lhsT=wsb[
                                        :, dc * 256 + m * 128: dc * 256 + (m + 1) * 128
                                    ],
                                    rhs=xt[(dc, half)][:, scq * 512:(scq + 1) * 512],
                                    start=(dc == 0),
                                    stop=(dc == 7),
                                )
                            # copy+bias+downcast: out = psum + b (per-partition)
                            nc.vector.tensor_scalar_add(
                                dst[m][:, sc * 512:(sc + 1) * 512],
                                ps[:],
                                bsb[:, m:m + 1],
                            )

        def emit_v_proj():
            xt = {}
            for half in range(2):
                for dc in range(8):
                    t = xs.tile([128, S // 2], bf16, name="xt", tag="xt")
                    nc.sync.dma_start(
                        out=t[:],
                        in_=xvT[
                            dc * 128:(dc + 1) * 128, half * 1024:(half + 1) * 1024
                        ],
                    )
                    xt[(dc, half)] = t
            for st in range(16):
                ps = ps_small.tile([128, 512], f32, name="ps", tag="ps")
                for dc in range(8):
                    nc.tensor.matmul(
                        ps[:, :260],
                        lhsT=xt[(dc, st // 8)][:, (st % 8) * 128:(st % 8 + 1) * 128],
                        rhs=wv_sb[:, dc * 260:(dc + 1) * 260],
                        start=(dc == 0),
                        stop=(dc == 7),
                    )
                # +bias (varies along free dim; bvp_sb is the DMA-broadcast
                # row), writes the ones column too (bvp has 1.0 at h*65+64).
                nc.vector.tensor_add(vh_sb[:, st, :], ps[:, :260], bvp_sb[:])

        if phase >= 1:
            emit_kq_proj(xkT, wk_sb, bk_sb, kt)
            emit_kq_proj(xqT, wq_sb, bq_sb, qt)
        if phase >= 2:
            emit_v_proj()

        # ---- attention ----
        # Normalize tail (bc broadcast / normalize-mul / O-proj) for qc is
        # pipelined one qc behind so the PE never head-of-line blocks on the
        # ln/exp reciprocal chain: PE stream per qc is
        #   [scores+attnV(qc,p0)] [bc(qc-1)] [scores+attnV(qc,p1)]
        #   [oproj(qc-1)] ...
        if phase >= 3:

            def emit_attention(qc, pair, cur_posb, cur_rcp):
                heads = (2 * pair, 2 * pair + 1)
                po = {}
                for h in heads:
                    po[h] = ps_av.tile([65, 512], f32, name="po", tag="po")
                ngroups = GK * (qc + 1)
                last_kc = 4 * qc + 3
                for g in range(ngroups):
                    exg = {}
                    for h in heads:
                        hr = slice(64 * (h % 2), 64 * (h % 2) + 64)
                        pss = ps_sc.tile([128, GK * QC], f32, name="pss", tag="pss")
                        for j in range(GK):
                            kc = GK * g + j
                            nc.tensor.matmul(
                                pss[:, j * QC:(j + 1) * QC],
                                lhsT=kt[pair][hr, kc * 128:(kc + 1) * 128],
                                rhs=qt[pair][hr, qc * QC:(qc + 1) * QC],
                                start=True,
                                stop=True,
                            )
                        ex = exps.tile([128, GK * QC], bf16, name="ex", tag="ex")
                        nc.scalar.activation(
                            ex[:], pss[:], mybir.ActivationFunctionType.Exp
                        )
                        dg = g - GK * qc  # diagonal subgroup index (0 or 1)
                        if dg >= 0:
                            nc.vector.tensor_mul(
                                ex[:],
                                ex[:],
                                cm_sb[:, dg * GK * QC:(dg + 1) * GK * QC],
                            )
                        exg[h] = ex
                    for h in heads:
                        for j in range(GK):
                            kc = GK * g + j
                            nc.tensor.matmul(
                                po[h][:],
                                lhsT=vh_sb[:, kc, h * 65:(h + 1) * 65],
                                rhs=exg[h][:, j * QC:(j + 1) * QC],
                                start=(kc == 0),
                                stop=(kc == last_kc),
                            )
                # denominator reciprocal on ScalarE (exp(-ln x); both funcs in
                # the natural_log_exp table set) + stage attn-out to SBUF bf16
                # so the po PSUM bank frees immediately.
                for h in heads:
                    lg = rcps.tile([65, 512], f32, name="lg", tag="lg", bufs=4)
                    nc.scalar.activation(
                        lg[64:65, :],
                        po[h][64:65, :],
                        mybir.ActivationFunctionType.Ln,
                    )
                    rcp = rcps.tile([65, 512], bf16, name="rcp", tag="rcp", bufs=8)
                    nc.scalar.activation(
                        rcp[64:65, :],
                        lg[64:65, :],
                        mybir.ActivationFunctionType.Exp,
                        scale=-1.0,
                    )
                    posb = rcps.tile([64, 512], bf16, name="posb", tag="posb", bufs=8)
                    with nc.allow_low_precision(reason="attn-out staged bf16"):
                        nc.vector.tensor_copy(posb[:, :], po[h][0:64, :])
                    cur_posb[h] = posb
                    cur_rcp[h] = rcp

            def emit_bc(qc, posb, rcp, bcs_out):
                for h in range(HPC):
                    bc = ps_small.tile([128, 512], f32, name="ps", tag="ps")
                    nc.tensor.matmul(
                        bc[0:64, :],
                        lhsT=ones_sb[64:65, :],
                        rhs=rcp[h][64:65, :],
                        start=True,
                        stop=True,
                    )
                    # DVE can't take two PSUM operands; stage via SBUF, then
                    # the normalize multiply runs on the otherwise-idle GpSimd.
                    bcs = rcps.tile([64, 512], bf16, name="bcs", tag="bcs", bufs=8)
                    nc.vector.tensor_copy(bcs[:, :], bc[0:64, :])
                    nc.gpsimd.tensor_mul(
                        outT[h][:, qc * QC:(qc + 1) * QC],
                        posb[h][:, :],
                        bcs[:, :],
                    )

            def emit_oproj(qc):
                for sti in range(4):
                    st = qc * 4 + sti
                    for ns in range(2):
                        ps = ps_small.tile([128, 512], f32, name="ps", tag="ps")
                        for h in range(HPC):
                            nc.tensor.matmul(
                                ps[:],
                                lhsT=outT[h][:, st * 128:(st + 1) * 128],
                                rhs=wo_sb[
                                    :, h * 1024 + ns * 512: h * 1024 + (ns + 1) * 512
                                ],
                                start=(h == 0),
                                stop=(h == 3),
                            )
                        ot = osb.tile([128, 512], f32, name="ot", tag="ot")
                        nc.vector.tensor_copy(ot[:], ps[:])
                        nc.sync.dma_start(
                            out=outp[
                                st * 128:(st + 1) * 128, ns * 512:(ns + 1) * 512
                            ],
                            in_=ot[:],
                        )

            pending = None  # (qc, posb{h}, rcp{h})
            for qc in range(NQC):
                cur_posb = {}
                cur_rcp = {}
                for pair in range(2):
                    emit_attention(qc, pair, cur_posb, cur_rcp)
                    if pair == 0 and pending is not None and phase >= 4:
                        emit_bc(pending[0], pending[1], pending[2], None)
                if pending is not None and phase >= 5:
                    emit_oproj(pending[0])
                pending = (qc, cur_posb, cur_rcp)
            if pending is not None and phase >= 4:
                emit_bc(pending[0], pending[1], pending[2], None)
                if phase >= 5:
                    emit_oproj(pending[0])

        if phase < 5:
            ot = osb.tile([128, 512], f32, name="ot", tag="ot")
            nc.vector.memset(ot[:], 0.0)
            nc.sync.dma_start(out=outp[0:128, 0:512], in_=ot[:])

    if split:
        _split_multi_waits(nc)
    return nc


_NC_CACHE = None


def _get_nc():
    global _NC_CACHE
    if _NC_CACHE is None:
        _NC_CACHE = _build_nc()
    return _NC_CACHE


def _swizzle_w(wT, block):
    """[1024, block*8?] no: wT [D, C] -> [128, 8*C] so that
    out[p, dc*C + j] = wT[dc*128 + p, j]."""
    dcs = wT.shape[0] // 128
    return np.ascontiguousarray(
        wT.reshape(dcs, 128, wT.shape[1]).transpose(1, 0, 2).reshape(128, -1)
    )


def _np_reference(q, k, v, mask, Wq, bq, Wk, bk, Wv, bv, Wo, bo):
    def split_heads(x):
        b, s, _ = x.shape
        return x.reshape(b, s, H, DK).transpose(0, 2, 1, 3)

    qh = split_heads(q @ Wq.T + bq)
    kh = split_heads(k @ Wk.T + bk)
    vh = split_heads(v @ Wv.T + bv)
    scores = np.einsum("bhqd,bhkd->bhqk", qh, kh) / np.sqrt(np.float32(DK))
    scores = np.where(mask, np.float32(-1e9), scores)
    scores = scores - scores.max(axis=-1, keepdims=True)
    e = np.exp(scores)
    attn = e / e.sum(axis=-1, keepdims=True)
    out = np.einsum("bhqk,bhkd->bhqd", attn, vh)
    out = out.transpose(0, 2, 1, 3).reshape(q.shape[0], -1, D)
    return (out @ Wo.T + bo).astype(np.float32)


def kernel(q, k, v, mask, Wq, bq, Wk, bk, Wv, bv, Wo, bo):
    import ml_dtypes

    bf16 = ml_dtypes.bfloat16

    q = np.asarray(q, np.float32)
    k = np.asarray(k, np.float32)
    v = np.asarray(v, np.float32)
    mask = np.asarray(mask, bool)
    Wq = np.asarray(Wq, np.float32)
    bq = np.asarray(bq, np.float32)
    Wk = np.asarray(Wk, np.float32)
    bk = np.asarray(bk, np.float32)
    Wv = np.asarray(Wv, np.float32)
    bv = np.asarray(bv, np.float32)
    Wo = np.asarray(Wo, np.float32)
    bo = np.asarray(bo, np.float32)

    causal = np.triu(np.ones((S, S), dtype=bool), k=1)
    if not np.array_equal(mask.reshape(S, S), causal):
        return _np_reference(q, k, v, mask, Wq, bq, Wk, bk, Wv, bv, Wo, bo)

    _install_ntff_hook()
    from concourse.bass_utils import run_bass_kernel_spmd

    nc = _get_nc()

    # causal keep-mask tiles for the 4 diagonal 128-blocks of a 512 q-chunk:
    # keep iff (128*j + kk) <= qq
    kk = np.arange(128)[:, None]
    qq = np.arange(QC)[None, :]
    cm = np.concatenate(
        [(128 * j + kk <= qq).astype(bf16) for j in range(4)], axis=1
    )  # [128, 2048]

    xT = {}
    for name, x in (("q", q), ("k", k), ("v", v)):
        xT[name] = [np.ascontiguousarray(x[b].T).astype(bf16) for b in range(B)]

    in_maps = []
    for c in range(N_CORES):
        b = c // 4
        g = c % 4
        hs = slice(g * HPC * DK, (g + 1) * HPC * DK)  # 256 rows of W, cols of Wo
        wq_c = _swizzle_w((SCALE * Wq[hs]).T.astype(bf16), 256)
        wk_c = _swizzle_w(Wk[hs].T.astype(bf16), 256)
        # V' with a zero weight column at h*65+64 (ones come via bias row)
        wvT = Wv[hs].T  # [1024, 256]
        wvp = np.zeros((D, 260), np.float32)
        for h in range(HPC):
            wvp[:, h * 65:h * 65 + 64] = wvT[:, h * 64:(h + 1) * 64]
        wv_c = _swizzle_w(wvp.astype(bf16), 260)
        # wo: (Wo.T)[hs, :] = Wo[:, hs].T  [256, 1024] -> [64, 4*1024]
        woT = np.ascontiguousarray(Wo[:, hs].T)
        wo_c = np.ascontiguousarray(
            woT.reshape(4, 64, 1024).transpose(1, 0, 2).reshape(64, 4096)
        ).astype(bf16)
        bq_c = np.ascontiguousarray(
            (SCALE * bq[hs]).reshape(2, 128).T.astype(np.float32)
        )
        bk_c = np.ascontiguousarray(bk[hs].reshape(2, 128).T.astype(np.float32))
        bvp_c = np.zeros((1, 260), np.float32)
        for h in range(HPC):
            bvp_c[0, h * 65:h * 65 + 64] = bv[hs][h * 64:(h + 1) * 64]
            bvp_c[0, h * 65 + 64] = 1.0
        in_maps.append(
            {
                "xqT": xT["q"][b],
                "xkT": xT["k"][b],
                "xvT": xT["v"][b],
                "wq": wq_c,
                "wk": wk_c,
                "wv": wv_c,
                "wo": wo_c,
                "bq": bq_c,
                "bk": bk_c,
                "bvp": bvp_c,
                "cmask": cm,
            }
        )

    trace = bool(os.environ.get("BASSMHA_TRACE"))
    res = run_bass_kernel_spmd(nc, in_maps, list(range(N_CORES)), trace=trace)
    kernel._last_exec_ns = res.exec_time_ns
    kernel._last_mean_exec_ns = res.mean_exec_time_ns

    out = np.zeros((B, S, D), np.float64)
    for c in range(N_CORES):
        out[c // 4] += res.results[c]["outp"].astype(np.float64)
    out += bo.astype(np.float64)
    return out.astype(np.float32)



# revision 11
# speedup vs baseline: 1.1593x; 1.1593x over previous
"""Multi-head attention (B=2, S=2048, D=1024, H=16, causal) on 8 Trainium2
NeuronCores via Bass/Tile.

Sharding: core c -> batch c//4, heads [4*(c%4), 4*(c%4)+4)  (DP over batch x
TP over heads).  QKV weights column-parallel, O row-parallel; the 4 partial
[S, D] outputs per batch are summed on the host (gather step), bias bo added
there too.

v2 dataflow (bf16 matmuls, fp32 PSUM accumulation), per core:
  - single sync-engine DMA queue in needed-by order: wk, bk, xk tiles woven
    with wq/cm/..., so the first K-proj matmul isn't stuck behind weights.
  - K proj -> kt [128 (2 heads x 64dk), S] x2 pairs; Q proj -> qt same.
  - scores for qc=0 are woven into the Q/V projection phase (Scalar exp
    starts ~25us in instead of ~55us).
  - scores computed transposed per kc block [128 k, 2, 512 q] covering BOTH
    heads of a pair; the two matmuls use partition rows 0:64 / 64:128 so the
    PE row-group tiling runs them concurrently.  ONE exp ACT covers both
    heads.  Diagonal blocks restrict to the causally valid q range
    (cols >= 128*j) for scores, exp, and attnV; the 128-wide diagonal strip
    gets the triangular keep-mask on DVE.
  - attnV: po[65, 512] per head; row 64 accumulates the softmax denominator
    via the ones column in V'.  kc loop is software-pipelined (attnV lags
    scores by 2) so the PE never waits on the exp chain.
  - normalize: DVE reciprocal of the denominator row -> rcpg[4, 512] bf16;
    one PE matmul (sel2) broadcasts a PAIR's two reciprocal rows to
    [128, 512]; gpsimd multiplies into outT2[pair] [128 (2 heads), S] bf16.
  - O proj: out[q, D] = sum_p outT2[p].T @ wo2[p] -- 2 accumulating matmuls
    per [128, 512] tile (full 128-deep contraction).
  - bc/normalize/O-proj work is queued as "fillers" popped one per kc
    iteration so the PE stream has no micro-idles (HAM stays at K=8/8).
"""

import os
import sys
import types
from collections import deque

import numpy as np

B, S, D, H = 2, 2048, 1024, 16
DK = D // H  # 64
N_CORES = 8
HPC = 4  # heads per core
SCALE = 1.0 / np.sqrt(np.float32(DK))  # folded into Wq/bq on host

QC = 512  # query block (free dim of scores matmuls)
KC = 128  # key block (partition dim of transposed scores)
NQC = S // QC  # 4
LAG = 2  # attnV trails scores by LAG kc-iterations


def _install_ntff_hook():
    """The image's antenv lacks axon_hooks; register the NTFF profile hook
    ourselves so run_bass_kernel_spmd(trace=True) works."""
    if "antenv.axon_hooks" in sys.modules:
        return
    try:
        mod = types.ModuleType("antenv.axon_hooks")
        state = {"hook": None}
        mod.set_axon_ntff_profile_hook = lambda h: state.__setitem__("hook", h)
        mod.get_axon_ntff_profile_hook = lambda: state["hook"]
        sys.modules["antenv.axon_hooks"] = mod
        from trn_agent_boot.trn_boot import _ntff_profile_via_ctypes

        mod.set_axon_ntff_profile_hook(
            _ntff_profile_via_ctypes("/opt/axon/libaxon_pjrt.so")
        )
    except Exception:
        sys.modules.pop("antenv.axon_hooks", None)


def _split_multi_waits(nc):
    """This walrus build accepts at most ONE sem wait per instruction; Tile
    packs several.  Split extras into preceding single-wait NOPs on the same
    engine (equivalent semantics: the engine blocks on them in order)."""
    import bass_rust

    cnt = 0
    for bbw in nc.main_func.blocks:
        bb = bbw.bb if hasattr(bbw, "bb") else bbw
        out = []
        changed = False
        for ins in bb.instructions:
            si = ins.sync_info
            if si is not None and len(si.on_wait) > 1:
                changed = True
                waits = list(si.on_wait)
                for w in waits[:-1]:
                    cnt += 1
                    nop = bass_rust.InstNoOp(name=f"I-wsp{cnt}", ins=[], outs=[])
                    nop.engine = ins.engine
                    nop.sync_info = bass_rust.SyncInfo(on_wait=[w], on_update=[])
                    out.append(nop)
                si.on_wait = [waits[-1]]
                ins.sync_info = si
            out.append(ins)
        if changed:
            bb.instructions = out
    return cnt


def _build_nc(split=True):
    from contextlib import ExitStack

    import concourse.bass as bass
    import concourse.tile as tile
    from concourse import mybir

    bf16 = mybir.dt.bfloat16
    f32 = mybir.dt.float32

    nc = bass.Bass()
    xqT = nc.declare_dram_parameter("xqT", [D, S], bf16, isOutput=False)
    xkT = nc.declare_dram_parameter("xkT", [D, S], bf16, isOutput=False)
    xvT = nc.declare_dram_parameter("xvT", [D, S], bf16, isOutput=False)
    wq = nc.declare_dram_parameter("wq", [128, 8 * 256], bf16, isOutput=False)
    wk = nc.declare_dram_parameter("wk", [128, 8 * 256], bf16, isOutput=False)
    wv = nc.declare_dram_parameter("wv", [128, 8 * 260], bf16, isOutput=False)
    wo2 = nc.declare_dram_parameter("wo2", [128, 2048], bf16, isOutput=False)
    bq = nc.declare_dram_parameter("bq", [128, 2], f32, isOutput=False)
    bk = nc.declare_dram_parameter("bk", [128, 2], f32, isOutput=False)
    bvp = nc.declare_dram_parameter("bvp", [1, 260], f32, isOutput=False)
    cm2 = nc.declare_dram_parameter("cm2", [128, 256], bf16, isOutput=False)
    sel2 = nc.declare_dram_parameter("sel2", [128, 256], bf16, isOutput=False)
    outp = nc.declare_dram_parameter("outp", [S, D], f32, isOutput=True)

    with tile.TileContext(nc) as tc, ExitStack() as ctx:
        consts = ctx.enter_context(tc.tile_pool(name="consts", bufs=1))
        xs = ctx.enter_context(tc.tile_pool(name="xs", bufs=24))
        acts = ctx.enter_context(tc.tile_pool(name="acts", bufs=1))
        exps = ctx.enter_context(tc.tile_pool(name="exps", bufs=12))
        posbp = ctx.enter_context(tc.tile_pool(name="posbp", bufs=4))
        bcsp = ctx.enter_context(tc.tile_pool(name="bcsp", bufs=2))
        osb = ctx.enter_context(tc.tile_pool(name="osb", bufs=4))
        ps_sc = ctx.enter_context(tc.tile_pool(name="ps_sc", bufs=2, space="PSUM"))
        ps1 = ctx.enter_context(tc.tile_pool(name="ps1", bufs=2, space="PSUM"))

        # ---- persistent activation tiles ----
        kt = [acts.tile([128, S], bf16, name=f"kt{m}", tag=f"kt{m}") for m in range(2)]
        qt = [acts.tile([128, S], bf16, name=f"qt{m}", tag=f"qt{m}") for m in range(2)]
        vh_sb = acts.tile([128, 16, 260], bf16, name="vh", tag="vh")
        outT2 = [
            acts.tile([128, S], bf16, name=f"outT2_{p}", tag=f"outT2_{p}")
            for p in range(2)
        ]
        # reciprocal rows live at partitions 32*(2*pair+hh); other partitions
        # stay at the memset value so the sel2 matmul contracts over zeros,
        # never garbage (0*NaN would poison the broadcast).
        rcpg = acts.tile([128, 512], bf16, name="rcpg", tag="rcpg")

        # ---- DMA issue, needed-by order, all on the sync-engine HW queue ----
        wk_sb = consts.tile([128, 8 * 256], bf16, name="wk_sb")
        nc.sync.dma_start(out=wk_sb[:], in_=wk[:])
        bk_sb = consts.tile([128, 2], f32, name="bk_sb")
        nc.sync.dma_start(out=bk_sb[:], in_=bk[:])

        xt = {}  # (input, dc, half) -> [128, 1024] bf16 tile

        def dma_x(src_, key, dc, half):
            t = xs.tile([128, S // 2], bf16, name="xt", tag="xt")
            nc.sync.dma_start(
                out=t[:],
                in_=src_[dc * 128:(dc + 1) * 128, half * 1024:(half + 1) * 1024],
            )
            xt[(key, dc, half)] = t

        for dc in range(8):
            dma_x(xkT, "k", dc, 0)
        wq_sb = consts.tile([128, 8 * 256], bf16, name="wq_sb")
        nc.sync.dma_start(out=wq_sb[:], in_=wq[:])
        bq_sb = consts.tile([128, 2], f32, name="bq_sb")
        nc.sync.dma_start(out=bq_sb[:], in_=bq[:])
        for dc in range(8):
            dma_x(xkT, "k", dc, 1)
        cm2_sb = consts.tile([128, 2, 128], bf16, name="cm2_sb")
        nc.sync.dma_start(out=cm2_sb[:], in_=cm2[:])
        sel2_sb = consts.tile([128, 256], bf16, name="sel2_sb")
        nc.sync.dma_start(out=sel2_sb[:], in_=sel2[:])
        nc.vector.memset(rcpg[:], 0.0)
        bvp_sb = consts.tile([128, 260], f32, name="bvp_sb")
        nc.sync.dma_start(out=bvp_sb[:], in_=bvp[:].to_broadcast((128, 260)))
        for half in range(2):
            for dc in range(8):
                dma_x(xqT, "q", dc, half)
        wv_sb = consts.tile([128, 8 * 260], bf16, name="wv_sb")
        nc.sync.dma_start(out=wv_sb[:], in_=wv[:])
        for half in range(2):
            for dc in range(8):
                dma_x(xvT, "v", dc, half)
        wo2_sb = consts.tile([128, 2048], bf16, name="wo2_sb")
        nc.sync.dma_start(out=wo2_sb[:], in_=wo2[:])

        # ---- projection helpers ----
        def kq_group(key, wsb, bsb, dst, m, sc):
            ps = ps1.tile([128, 512], f32, name="ps", tag="ps")
            for dc in range(8):
                nc.tensor.matmul(
                    ps[:],
                    lhsT=wsb[:, dc * 256 + m * 128: dc * 256 + (m + 1) * 128],
                    rhs=xt[(key, dc, sc // 2)][:, (sc % 2) * 512:(sc % 2) * 512 + 512],
                    start=(dc == 0),
                    stop=(dc == 7),
                )
            nc.vector.tensor_scalar_add(
                dst[m][:, sc * 512:(sc + 1) * 512], ps[:], bsb[:, m:m + 1]
            )

        def v_group(st):
            ps = ps1.tile([128, 512], f32, name="ps", tag="ps")
            for dc in range(8):
                nc.tensor.matmul(
                    ps[:, :260],
                    lhsT=xt[("v", dc, st // 8)][:, (st % 8) * 128:(st % 8 + 1) * 128],
                    rhs=wv_sb[:, dc * 260:(dc + 1) * 260],
                    start=(dc == 0),
                    stop=(dc == 7),
                )
            nc.vector.tensor_add(vh_sb[:, st, :], ps[:, :260], bvp_sb[:])

        # ---- attention helpers ----
        def emit_scores(qc, kc, pair):
            """scores + exp (+ causal mask) for one kc block, both heads of
            the pair.  Returns (ex tile, lo) for the matching attnV."""
            j = kc - 4 * qc  # diagonal sub-block index, or negative
            lo = 128 * j if j >= 0 else 0
            pss = ps_sc.tile([128, 2, 512], f32, name="pss", tag="pss")
            for hh in range(2):
                hr = slice(64 * hh, 64 * hh + 64)
                nc.tensor.matmul(
                    pss[:, hh, lo:],
                    lhsT=kt[pair][hr, kc * 128:(kc + 1) * 128],
                    rhs=qt[pair][hr, qc * QC + lo:(qc + 1) * QC],
                    start=True,
                    stop=True,
                )
            ex = exps.tile([128, 2, 512], bf16, name="ex", tag="ex")
            nc.scalar.activation(
                ex[:, :, lo:], pss[:, :, lo:], mybir.ActivationFunctionType.Exp
            )
            if j >= 0:
                # triangular keep-mask on the 128-wide diagonal strip
                nc.vector.tensor_mul(
                    ex[:, :, lo:lo + 128], ex[:, :, lo:lo + 128], cm2_sb[:]
                )
            return ex, lo

        def emit_attnv(qc, kc, pair, po, ex, lo):
            last = 4 * qc + 3
            for hh in range(2):
                h = 2 * pair + hh
                nc.tensor.matmul(
                    po[hh][:, lo:],
                    lhsT=vh_sb[:, kc, h * 65:(h + 1) * 65],
                    rhs=ex[:, hh, lo:],
                    start=(kc == 0),
                    stop=(kc == last),
                )

        def pair_end(qc, pair, po):
            """denominator reciprocal (DVE) + stage attn-out to SBUF bf16 so
            the po PSUM banks free immediately."""
            posb2 = posbp.tile([128, 512], bf16, name="posb2", tag="posb2")
            with nc.allow_low_precision(reason="rcp/attn-out staged bf16"):
                for hh in range(2):
                    r = 32 * (2 * pair + hh)
                    nc.vector.reciprocal(rcpg[r:r + 1, :], po[hh][64:65, :])
                    nc.vector.tensor_copy(
                        posb2[64 * hh:64 * hh + 64, :], po[hh][0:64, :]
                    )
            return posb2

        def make_bc_norm(qc, pair, posb2):
            def emit():
                bcps = ps1.tile([128, 512], f32, name="ps", tag="ps")
                nc.tensor.matmul(
                    bcps[:],
                    lhsT=sel2_sb[:, pair * 128:(pair + 1) * 128],
                    rhs=rcpg[:],
                    start=True,
                    stop=True,
                )
                bcs2 = bcsp.tile([128, 512], bf16, name="bcs2", tag="bcs2")
                nc.vector.tensor_copy(bcs2[:], bcps[:])
                nc.gpsimd.tensor_mul(
                    outT2[pair][:, qc * QC:(qc + 1) * QC], posb2[:], bcs2[:]
                )
            return emit

        def make_oproj(qc, g):
            def emit():
                sti, ns = g // 2, g % 2
                st = qc * 4 + sti
                ps = ps1.tile([128, 512], f32, name="ps", tag="ps")
                for p in range(2):
                    nc.tensor.matmul(
                        ps[:],
                        lhsT=outT2[p][:, st * 128:(st + 1) * 128],
                        rhs=wo2_sb[:, p * 1024 + ns * 512: p * 1024 + ns * 512 + 512],
                        start=(p == 0),
                        stop=(p == 1),
                    )
                ot = osb.tile([128, 512], f32, name="ot", tag="ot")
                nc.vector.tensor_copy(ot[:], ps[:])
                nc.sync.dma_start(
                    out=outp[st * 128:(st + 1) * 128, ns * 512:(ns + 1) * 512],
                    in_=ot[:],
                )
            return emit

        # ---- projections, with qc=0 scores woven in ----
        ex0 = {}  # (pair, kc) -> (ex, lo) for qc=0
        for half in range(2):
            for sc in (2 * half, 2 * half + 1):
                for m in range(2):
                    kq_group("k", wk_sb, bk_sb, kt, m, sc)
        for m in range(2):
            kq_group("q", wq_sb, bq_sb, qt, m, 0)
        for sc in range(1, 4):
            for pair in range(2):
                ex0[(pair, sc - 1)] = emit_scores(0, sc - 1, pair)
            for m in range(2):
                kq_group("q", wq_sb, bq_sb, qt, m, sc)
        v_group(0)
        v_group(1)
        for pair in range(2):
            ex0[(pair, 3)] = emit_scores(0, 3, pair)
        for st in range(2, 16):
            v_group(st)

        # ---- attention main loop ----
        fillers = deque()

        def pop_filler():
            if fillers:
                fillers.popleft()()

        for qc in range(NQC):
            for pair in range(2):
                po = [
                    ps1.tile([65, 512], f32, name=f"po{hh}", tag="po")
                    for hh in range(2)
                ]
                nkc = 4 * qc + 4
                if qc == 0:
                    for kc in range(nkc):
                        ex, lo = ex0[(pair, kc)]
                        emit_attnv(0, kc, pair, po, ex, lo)
                        if kc >= 1:
                            pop_filler()
                else:
                    meta = {}
                    for kc in range(nkc + LAG):
                        if kc < nkc:
                            meta[kc] = emit_scores(qc, kc, pair)
                        if kc >= LAG:
                            ex, lo = meta.pop(kc - LAG)
                            emit_attnv(qc, kc - LAG, pair, po, ex, lo)
                        if kc >= 1:
                            pop_filler()
                posb2 = pair_end(qc, pair, po)
                fillers.append(make_bc_norm(qc, pair, posb2))
                if pair == 1:
                    for g in range(8):
                        fillers.append(make_oproj(qc, g))
        while fillers:
            fillers.popleft()()

    if split:
        _split_multi_waits(nc)
    return nc


_NC_CACHE = None


def _get_nc():
    global _NC_CACHE
    if _NC_CACHE is None:
        _NC_CACHE = _build_nc()
    return _NC_CACHE


def _swizzle_w(wT, block):
    """wT [D, C] -> [128, 8*C] so that out[p, dc*C + j] = wT[dc*128 + p, j]."""
    dcs = wT.shape[0] // 128
    return np.ascontiguousarray(
        wT.reshape(dcs, 128, wT.shape[1]).transpose(1, 0, 2).reshape(128, -1)
    )


def _np_reference(q, k, v, mask, Wq, bq, Wk, bk, Wv, bv, Wo, bo):
    def split_heads(x):
        b, s, _ = x.shape
        return x.reshape(b, s, H, DK).transpose(0, 2, 1, 3)

    qh = split_heads(q @ Wq.T + bq)
    kh = split_heads(k @ Wk.T + bk)
    vh = split_heads(v @ Wv.T + bv)
    scores = np.einsum("bhqd,bhkd->bhqk", qh, kh) / np.sqrt(np.float32(DK))
    scores = np.where(mask, np.float32(-1e9), scores)
    scores = scores - scores.max(axis=-1, keepdims=True)
    e = np.exp(scores)
    attn = e / e.sum(axis=-1, keepdims=True)
    out = np.einsum("bhqk,bhkd->bhqd", attn, vh)
    out = out.transpose(0, 2, 1, 3).reshape(q.shape[0], -1, D)
    return (out @ Wo.T + bo).astype(np.float32)


def kernel(q, k, v, mask, Wq, bq, Wk, bk, Wv, bv, Wo, bo):
    import ml_dtypes

    bf16 = ml_dtypes.bfloat16

    q = np.asarray(q, np.float32)
    k = np.asarray(k, np.float32)
    v = np.asarray(v, np.float32)
    mask = np.asarray(mask, bool)
    Wq = np.asarray(Wq, np.float32)
    bq = np.asarray(bq, np.float32)
    Wk = np.asarray(Wk, np.float32)
    bk = np.asarray(bk, np.float32)
    Wv = np.asarray(Wv, np.float32)
    bv = np.asarray(bv, np.float32)
    Wo = np.asarray(Wo, np.float32)
    bo = np.asarray(bo, np.float32)

    causal = np.triu(np.ones((S, S), dtype=bool), k=1)
    if not np.array_equal(mask.reshape(S, S), causal):
        return _np_reference(q, k, v, mask, Wq, bq, Wk, bk, Wv, bv, Wo, bo)

    _install_ntff_hook()
    from concourse.bass_utils import run_bass_kernel_spmd

    nc = _get_nc()

    # triangular keep-mask for the 128-wide diagonal strip, doubled for the
    # two heads sharing one exp tile: keep iff kk <= qq
    kk = np.arange(128)[:, None]
    qq = np.arange(128)[None, :]
    tri = (kk <= qq).astype(bf16)
    cm2_np = np.concatenate([tri, tri], axis=1)  # [128, 256]

    # sel2[32*(2p+hh), p*128 + m] = 1 for m in the hh half: broadcast-select
    # the pair's two reciprocal rows (at partitions 0/32/64/96) onto 128
    sel2_np = np.zeros((128, 256), np.float32)
    for p in range(2):
        sel2_np[32 * (2 * p), p * 128:p * 128 + 64] = 1.0
        sel2_np[32 * (2 * p + 1), p * 128 + 64:p * 128 + 128] = 1.0
    sel2_np = sel2_np.astype(bf16)

    xT = {}
    for name, x in (("q", q), ("k", k), ("v", v)):
        xT[name] = [np.ascontiguousarray(x[b].T).astype(bf16) for b in range(B)]

    in_maps = []
    for c in range(N_CORES):
        b = c // 4
        g = c % 4
        hs = slice(g * HPC * DK, (g + 1) * HPC * DK)  # 256 rows of W, cols of Wo
        wq_c = _swizzle_w((SCALE * Wq[hs]).T.astype(bf16), 256)
        wk_c = _swizzle_w(Wk[hs].T.astype(bf16), 256)
        # V' with a zero weight column at h*65+64 (ones come via bias row)
        wvT = Wv[hs].T  # [1024, 256]
        wvp = np.zeros((D, 260), np.float32)
        for h in range(HPC):
            wvp[:, h * 65:h * 65 + 64] = wvT[:, h * 64:(h + 1) * 64]
        wv_c = _swizzle_w(wvp.astype(bf16), 260)
        # wo2: pair p columns hold (Wo[:, hs].T)[p*128:(p+1)*128, :]
        woT = np.ascontiguousarray(Wo[:, hs].T)  # [256, 1024]
        wo2_c = np.concatenate([woT[0:128], woT[128:256]], axis=1).astype(bf16)
        bq_c = np.ascontiguousarray(
            (SCALE * bq[hs]).reshape(2, 128).T.astype(np.float32)
        )
        bk_c = np.ascontiguousarray(bk[hs].reshape(2, 128).T.astype(np.float32))
        bvp_c = np.zeros((1, 260), np.float32)
        for h in range(HPC):
            bvp_c[0, h * 65:h * 65 + 64] = bv[hs][h * 64:(h + 1) * 64]
            bvp_c[0, h * 65 + 64] = 1.0
        in_maps.append(
            {
                "xqT": xT["q"][b],
                "xkT": xT["k"][b],
                "xvT": xT["v"][b],
                "wq": wq_c,
                "wk": wk_c,
                "wv": wv_c,
                "wo2": wo2_c,
                "bq": bq_c,
                "bk": bk_c,
                "bvp": bvp_c,
                "cm2": cm2_np,
                "sel2": sel2_np,
            }
        )

    trace = bool(os.environ.get("BASSMHA_TRACE"))
    res = run_bass_kernel_spmd(nc, in_maps, list(range(N_CORES)), trace=trace)
    kernel._last_exec_ns = res.exec_time_ns
    kernel._last_mean_exec_ns = res.mean_exec_time_ns

    out = np.zeros((B, S, D), np.float64)
    for c in range(N_CORES):
        out[c // 4] += res.results[c]["outp"].astype(np.float64)
    out += bo.astype(np.float64)
    return out.astype(np.float32)


# revision 19
# speedup vs baseline: 1.1773x; 1.0155x over previous
"""Multi-head attention (B=2, S=2048, D=1024, H=16, causal) on 8 Trainium2
NeuronCores via Bass/Tile.

Sharding: core c -> batch c//4, heads [4*(c%4), 4*(c%4)+4)  (DP over batch x
TP over heads).  QKV weights column-parallel, O row-parallel; the 4 partial
[S, D] outputs per batch are summed on the host (gather step), bias bo added
there too.

v2 dataflow (bf16 matmuls, fp32 PSUM accumulation), per core:
  - single sync-engine DMA queue in needed-by order: wk, bk, xk tiles woven
    with wq/cm/..., so the first K-proj matmul isn't stuck behind weights.
  - K proj -> kt [128 (2 heads x 64dk), S] x2 pairs; Q proj -> qt same.
  - scores for qc=0 are woven into the Q/V projection phase (Scalar exp
    starts ~25us in instead of ~55us).
  - scores computed transposed per kc block [128 k, 2, 512 q] covering BOTH
    heads of a pair; the two matmuls use partition rows 0:64 / 64:128 so the
    PE row-group tiling runs them concurrently.  ONE exp ACT covers both
    heads.  Diagonal blocks restrict to the causally valid q range
    (cols >= 128*j) for scores, exp, and attnV; the 128-wide diagonal strip
    gets the triangular keep-mask on DVE.
  - attnV: po[65, 512] per head; row 64 accumulates the softmax denominator
    via the ones column in V'.  kc loop is software-pipelined (attnV lags
    scores by 2) so the PE never waits on the exp chain.
  - normalize: DVE reciprocal of the denominator row -> rcpg[4, 512] bf16;
    one PE matmul (sel2) broadcasts a PAIR's two reciprocal rows to
    [128, 512]; gpsimd multiplies into outT2[pair] [128 (2 heads), S] bf16.
  - O proj: out[q, D] = sum_p outT2[p].T @ wo2[p] -- 2 accumulating matmuls
    per [128, 512] tile (full 128-deep contraction).
  - bc/normalize/O-proj work is queued as "fillers" popped one per kc
    iteration so the PE stream has no micro-idles (HAM stays at K=8/8).
"""

import os
import sys
import types
from collections import deque

import numpy as np

B, S, D, H = 2, 2048, 1024, 16
DK = D // H  # 64
N_CORES = 8
HPC = 4  # heads per core
SCALE = 1.0 / np.sqrt(np.float32(DK))  # folded into Wq/bq on host

QC = 512  # query block (free dim of scores matmuls)
KC = 128  # key block (partition dim of transposed scores)
NQC = S // QC  # 4
LAG = 2  # attnV trails scores by LAG kc-iterations


def _install_ntff_hook():
    """The image's antenv lacks axon_hooks; register the NTFF profile hook
    ourselves so run_bass_kernel_spmd(trace=True) works."""
    if "antenv.axon_hooks" in sys.modules:
        return
    try:
        mod = types.ModuleType("antenv.axon_hooks")
        state = {"hook": None}
        mod.set_axon_ntff_profile_hook = lambda h: state.__setitem__("hook", h)
        mod.get_axon_ntff_profile_hook = lambda: state["hook"]
        sys.modules["antenv.axon_hooks"] = mod
        from trn_agent_boot.trn_boot import _ntff_profile_via_ctypes

        mod.set_axon_ntff_profile_hook(
            _ntff_profile_via_ctypes("/opt/axon/libaxon_pjrt.so")
        )
    except Exception:
        sys.modules.pop("antenv.axon_hooks", None)


def _split_multi_waits(nc):
    """This walrus build accepts at most ONE sem wait per instruction; Tile
    packs several.  Split extras into preceding single-wait NOPs on the same
    engine (equivalent semantics: the engine blocks on them in order)."""
    import bass_rust

    cnt = 0
    for bbw in nc.main_func.blocks:
        bb = bbw.bb if hasattr(bbw, "bb") else bbw
        out = []
        changed = False
        for ins in bb.instructions:
            si = ins.sync_info
            if si is not None and len(si.on_wait) > 1:
                changed = True
                waits = list(si.on_wait)
                for w in waits[:-1]:
                    cnt += 1
                    nop = bass_rust.InstNoOp(name=f"I-wsp{cnt}", ins=[], outs=[])
                    nop.engine = ins.engine
                    nop.sync_info = bass_rust.SyncInfo(on_wait=[w], on_update=[])
                    out.append(nop)
                si.on_wait = [waits[-1]]
                ins.sync_info = si
            out.append(ins)
        if changed:
            bb.instructions = out
    return cnt


def _build_nc(split=True):
    from contextlib import ExitStack

    import concourse.bass as bass
    import concourse.tile as tile
    from concourse import mybir

    bf16 = mybir.dt.bfloat16
    f32 = mybir.dt.float32

    nc = bass.Bass()
    xqT = nc.declare_dram_parameter("xqT", [D, S], bf16, isOutput=False)
    xkT = nc.declare_dram_parameter("xkT", [D, S], bf16, isOutput=False)
    xvT = nc.declare_dram_parameter("xvT", [D, S], bf16, isOutput=False)
    wq = nc.declare_dram_parameter("wq", [128, 8 * 256], bf16, isOutput=False)
    wk = nc.declare_dram_parameter("wk", [128, 8 * 256], bf16, isOutput=False)
    wv = nc.declare_dram_parameter("wv", [128, 8 * 260], bf16, isOutput=False)
    wo2 = nc.declare_dram_parameter("wo2", [128, 2048], bf16, isOutput=False)
    bq = nc.declare_dram_parameter("bq", [128, 2], f32, isOutput=False)
    bk = nc.declare_dram_parameter("bk", [128, 2], f32, isOutput=False)
    bvp = nc.declare_dram_parameter("bvp", [1, 260], f32, isOutput=False)
    cm2 = nc.declare_dram_parameter("cm2", [128, 256], bf16, isOutput=False)
    sel2 = nc.declare_dram_parameter("sel2", [128, 256], bf16, isOutput=False)
    outp = nc.declare_dram_parameter("outp", [S, D], f32, isOutput=True)

    with tile.TileContext(nc) as tc, ExitStack() as ctx:
        consts = ctx.enter_context(tc.tile_pool(name="consts", bufs=1))
        xs = ctx.enter_context(tc.tile_pool(name="xs", bufs=24))
        acts = ctx.enter_context(tc.tile_pool(name="acts", bufs=1))
        exps = ctx.enter_context(tc.tile_pool(name="exps", bufs=26))
        posbp = ctx.enter_context(tc.tile_pool(name="posbp", bufs=4))
        scrp = ctx.enter_context(tc.tile_pool(name="scrp", bufs=4))
        bcsp = ctx.enter_context(tc.tile_pool(name="bcsp", bufs=2))
        osb = ctx.enter_context(tc.tile_pool(name="osb", bufs=4))
        ps_sc = ctx.enter_context(tc.tile_pool(name="ps_sc", bufs=2, space="PSUM"))
        ps1 = ctx.enter_context(tc.tile_pool(name="ps1", bufs=2, space="PSUM"))

        # ---- persistent activation tiles ----
        kt = [acts.tile([128, S], bf16, name=f"kt{m}", tag=f"kt{m}") for m in range(2)]
        qt = [acts.tile([128, S], bf16, name=f"qt{m}", tag=f"qt{m}") for m in range(2)]
        vh_sb = acts.tile([128, 16, 260], bf16, name="vh", tag="vh")
        outT2 = [
            acts.tile([128, S], bf16, name=f"outT2_{p}", tag=f"outT2_{p}")
            for p in range(2)
        ]
        # reciprocal rows live at partitions 32*(2*pair+hh); other partitions
        # stay at the memset value so the sel2 matmul contracts over zeros,
        # never garbage (0*NaN would poison the broadcast).
        rcpg = acts.tile([128, 512], bf16, name="rcpg", tag="rcpg")

        # ---- DMA issue, needed-by order, all on the sync-engine HW queue ----
        wk_sb = consts.tile([128, 8 * 256], bf16, name="wk_sb")
        nc.sync.dma_start(out=wk_sb[:], in_=wk[:])
        bk_sb = consts.tile([128, 2], f32, name="bk_sb")
        nc.sync.dma_start(out=bk_sb[:], in_=bk[:])

        xt = {}  # (input, dc, half) -> [128, 1024] bf16 tile

        def dma_x(src_, key, dc, half):
            t = xs.tile([128, S // 2], bf16, name="xt", tag="xt")
            nc.sync.dma_start(
                out=t[:],
                in_=src_[dc * 128:(dc + 1) * 128, half * 1024:(half + 1) * 1024],
            )
            xt[(key, dc, half)] = t

        for dc in range(8):
            dma_x(xkT, "k", dc, 0)
        wq_sb = consts.tile([128, 8 * 256], bf16, name="wq_sb")
        nc.sync.dma_start(out=wq_sb[:], in_=wq[:])
        bq_sb = consts.tile([128, 2], f32, name="bq_sb")
        nc.sync.dma_start(out=bq_sb[:], in_=bq[:])
        for dc in range(8):
            dma_x(xkT, "k", dc, 1)
        cm2_sb = consts.tile([128, 2, 128], bf16, name="cm2_sb")
        nc.sync.dma_start(out=cm2_sb[:], in_=cm2[:])
        sel2_sb = consts.tile([128, 256], bf16, name="sel2_sb")
        nc.sync.dma_start(out=sel2_sb[:], in_=sel2[:])
        nc.vector.memset(rcpg[:], 0.0)
        bvp_sb = consts.tile([128, 260], f32, name="bvp_sb")
        nc.sync.dma_start(out=bvp_sb[:], in_=bvp[:].to_broadcast((128, 260)))
        for half in range(2):
            for dc in range(8):
                dma_x(xqT, "q", dc, half)
        wv_sb = consts.tile([128, 8 * 260], bf16, name="wv_sb")
        nc.sync.dma_start(out=wv_sb[:], in_=wv[:])
        for half in range(2):
            for dc in range(8):
                dma_x(xvT, "v", dc, half)
        wo2_sb = consts.tile([128, 2048], bf16, name="wo2_sb")
        nc.sync.dma_start(out=wo2_sb[:], in_=wo2[:])

        # ---- projection helpers ----
        # dc is the OUTER loop so the first matmul only waits on the first
        # input tile (the PE paces with the DMA stream instead of stalling
        # for all 8 chunks).
        def kq_sc(key, wsb, bsb, dst, sc):
            ps = [ps1.tile([128, 512], f32, name="ps", tag="ps") for _ in range(2)]
            for dc in range(8):
                for m in range(2):
                    nc.tensor.matmul(
                        ps[m][:],
                        lhsT=wsb[:, dc * 256 + m * 128: dc * 256 + (m + 1) * 128],
                        rhs=xt[(key, dc, sc // 2)][
                            :, (sc % 2) * 512:(sc % 2) * 512 + 512
                        ],
                        start=(dc == 0),
                        stop=(dc == 7),
                    )
            for m in range(2):
                nc.vector.tensor_scalar_add(
                    dst[m][:, sc * 512:(sc + 1) * 512], ps[m][:], bsb[:, m:m + 1]
                )

        def v_stpair(sp):
            sts = (2 * sp, 2 * sp + 1)
            ps = [ps1.tile([128, 512], f32, name="ps", tag="ps") for _ in range(2)]
            for dc in range(8):
                for i, st in enumerate(sts):
                    nc.tensor.matmul(
                        ps[i][:, :260],
                        lhsT=xt[("v", dc, st // 8)][
                            :, (st % 8) * 128:(st % 8 + 1) * 128
                        ],
                        rhs=wv_sb[:, dc * 260:(dc + 1) * 260],
                        start=(dc == 0),
                        stop=(dc == 7),
                    )
            for i, st in enumerate(sts):
                nc.vector.tensor_add(vh_sb[:, st, :], ps[i][:, :260], bvp_sb[:])

        # ---- attention helpers ----
        def emit_scores(qc, kc, pair):
            """scores + exp (+ causal mask) for one kc block, both heads of
            the pair.  Returns (ex tile, lo) for the matching attnV."""
            j = kc - 4 * qc  # diagonal sub-block index, or negative
            lo = 128 * j if j >= 0 else 0
            pss = ps_sc.tile([128, 2, 512], f32, name="pss", tag="pss")
            for hh in range(2):
                hr = slice(64 * hh, 64 * hh + 64)
                nc.tensor.matmul(
                    pss[:, hh, lo:],
                    lhsT=kt[pair][hr, kc * 128:(kc + 1) * 128],
                    rhs=qt[pair][hr, qc * QC + lo:(qc + 1) * QC],
                    start=True,
                    stop=True,
                )
            ex = exps.tile([128, 2, 512], bf16, name="ex", tag="ex")
            nc.scalar.activation(
                ex[:, :, lo:], pss[:, :, lo:], mybir.ActivationFunctionType.Exp
            )
            if j >= 0:
                # triangular keep-mask on the 128-wide diagonal strip
                nc.vector.tensor_mul(
                    ex[:, :, lo:lo + 128], ex[:, :, lo:lo + 128], cm2_sb[:]
                )
            return ex, lo

        def emit_attnv(qc, kc, pair, po, ex, lo):
            last = 4 * qc + 3
            for hh in range(2):
                h = 2 * pair + hh
                nc.tensor.matmul(
                    po[hh][:, lo:],
                    lhsT=vh_sb[:, kc, h * 65:(h + 1) * 65],
                    rhs=ex[:, hh, lo:],
                    start=(kc == 0),
                    stop=(kc == last),
                )

        def pair_end(qc, pair, po):
            """free the po PSUM banks fast: reciprocal (fast-approx, DVE) of
            the denominator rows and attn-out staging (ACT copies) run in
            parallel; the f32->bf16 cast of the reciprocal happens off the
            critical path on gpsimd."""
            posb2 = posbp.tile([128, 512], bf16, name="posb2", tag="posb2")
            for hh in range(2):
                r = 32 * (2 * pair + hh)
                scr = scrp.tile([1, 512], f32, name="scr", tag="scr")
                nc.vector.reciprocal(scr[:], po[hh][64:65, :])
                nc.scalar.copy(posb2[64 * hh:64 * hh + 64, :], po[hh][0:64, :])
                with nc.allow_low_precision(reason="rcp bf16, same as matmul"):
                    nc.vector.tensor_copy(rcpg[r:r + 1, :], scr[:])
            return posb2

        def make_bc_norm(qc, pair, posb2):
            def emit():
                bcps = ps1.tile([128, 512], f32, name="ps", tag="ps")
                nc.tensor.matmul(
                    bcps[:],
                    lhsT=sel2_sb[:, pair * 128:(pair + 1) * 128],
                    rhs=rcpg[:],
                    start=True,
                    stop=True,
                )
                bcs2 = bcsp.tile([128, 512], bf16, name="bcs2", tag="bcs2")
                nc.vector.tensor_copy(bcs2[:], bcps[:])
                nc.gpsimd.tensor_mul(
                    outT2[pair][:, qc * QC:(qc + 1) * QC], posb2[:], bcs2[:]
                )
            return emit

        def make_oproj(qc, g):
            def emit():
                sti, ns = g // 2, g % 2
                st = qc * 4 + sti
                ps = ps1.tile([128, 512], f32, name="ps", tag="ps")
                for p in range(2):
                    nc.tensor.matmul(
                        ps[:],
                        lhsT=outT2[p][:, st * 128:(st + 1) * 128],
                        rhs=wo2_sb[:, p * 1024 + ns * 512: p * 1024 + ns * 512 + 512],
                        start=(p == 0),
                        stop=(p == 1),
                    )
                ot = osb.tile([128, 512], f32, name="ot", tag="ot")
                nc.vector.tensor_copy(ot[:], ps[:])
                nc.sync.dma_start(
                    out=outp[st * 128:(st + 1) * 128, ns * 512:(ns + 1) * 512],
                    in_=ot[:],
                )
            return emit

        # ---- projections, with qc=0 and qc=1 scores woven in ----
        pre = {}  # (qc, pair, kc) -> (ex, lo)
        for sc in range(4):
            kq_sc("k", wk_sb, bk_sb, kt, sc)
        kq_sc("q", wq_sb, bq_sb, qt, 0)
        for sc in range(1, 4):
            for pair in range(2):
                pre[(0, pair, sc - 1)] = emit_scores(0, sc - 1, pair)
            kq_sc("q", wq_sb, bq_sb, qt, sc)
        for pair in range(2):
            pre[(0, pair, 3)] = emit_scores(0, 3, pair)
        for sp in range(8):
            v_stpair(sp)
            for pair in range(2):
                pre[(1, pair, sp)] = emit_scores(1, sp, pair)

        # ---- attention main loop ----
        fillers = deque()

        def pop_filler():
            if fillers:
                fillers.popleft()()

        for qc in range(NQC):
            for pair in range(2):
                po = [
                    ps1.tile([65, 512], f32, name=f"po{hh}", tag="po")
                    for hh in range(2)
                ]
                nkc = 4 * qc + 4
                if qc <= 1:
                    for kc in range(nkc):
                        ex, lo = pre.pop((qc, pair, kc))
                        emit_attnv(qc, kc, pair, po, ex, lo)
                        if kc >= 1:
                            pop_filler()
                else:
                    meta = {}
                    for kc in range(nkc + LAG):
                        if kc < nkc:
                            meta[kc] = emit_scores(qc, kc, pair)
                        if kc >= LAG:
                            ex, lo = meta.pop(kc - LAG)
                            emit_attnv(qc, kc - LAG, pair, po, ex, lo)
                        if kc >= 1:
                            pop_filler()
                posb2 = pair_end(qc, pair, po)
                fillers.append(make_bc_norm(qc, pair, posb2))
                if pair == 1:
                    for g in range(8):
                        fillers.append(make_oproj(qc, g))
        while fillers:
            fillers.popleft()()

    if split:
        _split_multi_waits(nc)
    return nc


_NC_CACHE = None


def _get_nc():
    global _NC_CACHE
    if _NC_CACHE is None:
        _NC_CACHE = _build_nc()
    return _NC_CACHE


def _swizzle_w(wT, block):
    """wT [D, C] -> [128, 8*C] so that out[p, dc*C + j] = wT[dc*128 + p, j]."""
    dcs = wT.shape[0] // 128
    return np.ascontiguousarray(
        wT.reshape(dcs, 128, wT.shape[1]).transpose(1, 0, 2).reshape(128, -1)
    )


def _np_reference(q, k, v, mask, Wq, bq, Wk, bk, Wv, bv, Wo, bo):
    def split_heads(x):
        b, s, _ = x.shape
        return x.reshape(b, s, H, DK).transpose(0, 2, 1, 3)

    qh = split_heads(q @ Wq.T + bq)
    kh = split_heads(k @ Wk.T + bk)
    vh = split_heads(v @ Wv.T + bv)
    scores = np.einsum("bhqd,bhkd->bhqk", qh, kh) / np.sqrt(np.float32(DK))
    scores = np.where(mask, np.float32(-1e9), scores)
    scores = scores - scores.max(axis=-1, keepdims=True)
    e = np.exp(scores)
    attn = e / e.sum(axis=-1, keepdims=True)
    out = np.einsum("bhqk,bhkd->bhqd", attn, vh)
    out = out.transpose(0, 2, 1, 3).reshape(q.shape[0], -1, D)
    return (out @ Wo.T + bo).astype(np.float32)


def kernel(q, k, v, mask, Wq, bq, Wk, bk, Wv, bv, Wo, bo):
    import ml_dtypes

    bf16 = ml_dtypes.bfloat16

    q = np.asarray(q, np.float32)
    k = np.asarray(k, np.float32)
    v = np.asarray(v, np.float32)
    mask = np.asarray(mask, bool)
    Wq = np.asarray(Wq, np.float32)
    bq = np.asarray(bq, np.float32)
    Wk = np.asarray(Wk, np.float32)
    bk = np.asarray(bk, np.float32)
    Wv = np.asarray(Wv, np.float32)
    bv = np.asarray(bv, np.float32)
    Wo = np.asarray(Wo, np.float32)
    bo = np.asarray(bo, np.float32)

    causal = np.triu(np.ones((S, S), dtype=bool), k=1)
    if not np.array_equal(mask.reshape(S, S), causal):
        return _np_reference(q, k, v, mask, Wq, bq, Wk, bk, Wv, bv, Wo, bo)

    _install_ntff_hook()
    from concourse.bass_utils import run_bass_kernel_spmd

    nc = _get_nc()

    # triangular keep-mask for the 128-wide diagonal strip, doubled for the
    # two heads sharing one exp tile: keep iff kk <= qq
    kk = np.arange(128)[:, None]
    qq = np.arange(128)[None, :]
    tri = (kk <= qq).astype(bf16)
    cm2_np = np.concatenate([tri, tri], axis=1)  # [128, 256]

    # sel2[32*(2p+hh), p*128 + m] = 1 for m in the hh half: broadcast-select
    # the pair's two reciprocal rows (at partitions 0/32/64/96) onto 128
    sel2_np = np.zeros((128, 256), np.float32)
    for p in range(2):
        sel2_np[32 * (2 * p), p * 128:p * 128 + 64] = 1.0
        sel2_np[32 * (2 * p + 1), p * 128 + 64:p * 128 + 128] = 1.0
    sel2_np = sel2_np.astype(bf16)

    xT = {}
    for name, x in (("q", q), ("k", k), ("v", v)):
        xT[name] = [np.ascontiguousarray(x[b].T).astype(bf16) for b in range(B)]

    in_maps = []
    for c in range(N_CORES):
        b = c // 4
        g = c % 4
        hs = slice(g * HPC * DK, (g + 1) * HPC * DK)  # 256 rows of W, cols of Wo
        wq_c = _swizzle_w((SCALE * Wq[hs]).T.astype(bf16), 256)
        wk_c = _swizzle_w(Wk[hs].T.astype(bf16), 256)
        # V' with a zero weight column at h*65+64 (ones come via bias row)
        wvT = Wv[hs].T  # [1024, 256]
        wvp = np.zeros((D, 260), np.float32)
        for h in range(HPC):
            wvp[:, h * 65:h * 65 + 64] = wvT[:, h * 64:(h + 1) * 64]
        wv_c = _swizzle_w(wvp.astype(bf16), 260)
        # wo2: pair p columns hold (Wo[:, hs].T)[p*128:(p+1)*128, :]
        woT = np.ascontiguousarray(Wo[:, hs].T)  # [256, 1024]
        wo2_c = np.concatenate([woT[0:128], woT[128:256]], axis=1).astype(bf16)
        bq_c = np.ascontiguousarray(
            (SCALE * bq[hs]).reshape(2, 128).T.astype(np.float32)
        )
        bk_c = np.ascontiguousarray(bk[hs].reshape(2, 128).T.astype(np.float32))
        bvp_c = np.zeros((1, 260), np.float32)
        for h in range(HPC):
            bvp_c[0, h * 65:h * 65 + 64] = bv[hs][h * 64:(h + 1) * 64]
            bvp_c[0, h * 65 + 64] = 1.0
        in_maps.append(
            {
                "xqT": xT["q"][b],
                "xkT": xT["k"][b],
                "xvT": xT["v"][b],
                "wq": wq_c,
                "wk": wk_c,
                "wv": wv_c,
                "wo2": wo2_c,
                "bq": bq_c,
                "bk": bk_c,
                "bvp": bvp_c,
                "cm2": cm2_np,
                "sel2": sel2_np,
            }
        )

    trace = bool(os.environ.get("BASSMHA_TRACE"))
    res = run_bass_kernel_spmd(nc, in_maps, list(range(N_CORES)), trace=trace)
    kernel._last_exec_ns = res.exec_time_ns
    kernel._last_mean_exec_ns = res.mean_exec_time_ns

    out = np.zeros((B, S, D), np.float64)
    for c in range(N_CORES):
        out[c // 4] += res.results[c]["outp"].astype(np.float64)
    out += bo.astype(np.float64)
    return out.astype(np.float32)


# revision 20
# speedup vs baseline: 1.4405x; 1.2236x over previous
"""Multi-head attention (B=2, S=2048, D=1024, H=16, causal) on 8 Trainium2
NeuronCores via Bass/Tile.

Sharding: core c -> batch c//4, heads [4*(c%4), 4*(c%4)+4)  (DP over batch x
TP over heads).  QKV weights column-parallel, O row-parallel; the 4 partial
[S, D] outputs per batch are summed on the host (gather step), bias bo added
there too.

v2 dataflow (bf16 matmuls, fp32 PSUM accumulation), per core:
  - single sync-engine DMA queue in needed-by order: wk, bk, xk tiles woven
    with wq/cm/..., so the first K-proj matmul isn't stuck behind weights.
  - K proj -> kt [128 (2 heads x 64dk), S] x2 pairs; Q proj -> qt same.
  - scores for qc=0 are woven into the Q/V projection phase (Scalar exp
    starts ~25us in instead of ~55us).
  - scores computed transposed per kc block [128 k, 2, 512 q] covering BOTH
    heads of a pair; the two matmuls use partition rows 0:64 / 64:128 so the
    PE row-group tiling runs them concurrently.  ONE exp ACT covers both
    heads.  Diagonal blocks restrict to the causally valid q range
    (cols >= 128*j) for scores, exp, and attnV; the 128-wide diagonal strip
    gets the triangular keep-mask on DVE.
  - attnV: po[65, 512] per head; row 64 accumulates the softmax denominator
    via the ones column in V'.  kc loop is software-pipelined (attnV lags
    scores by 2) so the PE never waits on the exp chain.
  - normalize: DVE reciprocal of the denominator row -> rcpg[4, 512] bf16;
    one PE matmul (sel2) broadcasts a PAIR's two reciprocal rows to
    [128, 512]; gpsimd multiplies into outT2[pair] [128 (2 heads), S] bf16.
  - O proj: out[q, D] = sum_p outT2[p].T @ wo2[p] -- 2 accumulating matmuls
    per [128, 512] tile (full 128-deep contraction).
  - bc/normalize/O-proj work is queued as "fillers" popped one per kc
    iteration so the PE stream has no micro-idles (HAM stays at K=8/8).
"""

import os
import sys
import types
from collections import deque

import numpy as np

B, S, D, H = 2, 2048, 1024, 16
DK = D // H  # 64
N_CORES = 8
HPC = 4  # heads per core
SCALE = 1.0 / np.sqrt(np.float32(DK))  # folded into Wq/bq on host

QC = 512  # query block (free dim of scores matmuls)
KC = 128  # key block (partition dim of transposed scores)
NQC = S // QC  # 4
LAG = 2  # attnV trails scores by LAG kc-iterations


def _install_ntff_hook():
    """The image's antenv lacks axon_hooks; register the NTFF profile hook
    ourselves so run_bass_kernel_spmd(trace=True) works."""
    if "antenv.axon_hooks" in sys.modules:
        return
    try:
        mod = types.ModuleType("antenv.axon_hooks")
        state = {"hook": None}
        mod.set_axon_ntff_profile_hook = lambda h: state.__setitem__("hook", h)
        mod.get_axon_ntff_profile_hook = lambda: state["hook"]
        sys.modules["antenv.axon_hooks"] = mod
        from trn_agent_boot.trn_boot import _ntff_profile_via_ctypes

        mod.set_axon_ntff_profile_hook(
            _ntff_profile_via_ctypes("/opt/axon/libaxon_pjrt.so")
        )
    except Exception:
        sys.modules.pop("antenv.axon_hooks", None)


def _split_multi_waits(nc):
    """This walrus build accepts at most ONE sem wait per instruction; Tile
    packs several.  Split extras into preceding single-wait NOPs on the same
    engine (equivalent semantics: the engine blocks on them in order)."""
    import bass_rust

    cnt = 0
    for bbw in nc.main_func.blocks:
        bb = bbw.bb if hasattr(bbw, "bb") else bbw
        out = []
        changed = False
        for ins in bb.instructions:
            si = ins.sync_info
            if si is not None and len(si.on_wait) > 1:
                changed = True
                waits = list(si.on_wait)
                for w in waits[:-1]:
                    cnt += 1
                    nop = bass_rust.InstNoOp(name=f"I-wsp{cnt}", ins=[], outs=[])
                    nop.engine = ins.engine
                    nop.sync_info = bass_rust.SyncInfo(on_wait=[w], on_update=[])
                    out.append(nop)
                si.on_wait = [waits[-1]]
                ins.sync_info = si
            out.append(ins)
        if changed:
            bb.instructions = out
    return cnt


def _build_nc(split=True):
    from contextlib import ExitStack

    import concourse.bass as bass
    import concourse.tile as tile
    from concourse import mybir

    bf16 = mybir.dt.bfloat16
    f32 = mybir.dt.float32

    nc = bass.Bass()
    xqT = nc.declare_dram_parameter("xqT", [D, S], bf16, isOutput=False)
    xkT = nc.declare_dram_parameter("xkT", [D, S], bf16, isOutput=False)
    xvT = nc.declare_dram_parameter("xvT", [D, S], bf16, isOutput=False)
    wq = nc.declare_dram_parameter("wq", [128, 8 * 256], bf16, isOutput=False)
    wk = nc.declare_dram_parameter("wk", [128, 8 * 256], bf16, isOutput=False)
    wv = nc.declare_dram_parameter("wv", [128, 8 * 260], bf16, isOutput=False)
    wo2 = nc.declare_dram_parameter("wo2", [128, 2048], bf16, isOutput=False)
    bq = nc.declare_dram_parameter("bq", [128, 2], f32, isOutput=False)
    bk = nc.declare_dram_parameter("bk", [128, 2], f32, isOutput=False)
    bvp = nc.declare_dram_parameter("bvp", [1, 260], f32, isOutput=False)
    cm2 = nc.declare_dram_parameter("cm2", [128, 256], bf16, isOutput=False)
    sel2 = nc.declare_dram_parameter("sel2", [128, 256], bf16, isOutput=False)
    outp = nc.declare_dram_parameter("outp", [S, D], f32, isOutput=True)

    with tile.TileContext(nc) as tc, ExitStack() as ctx:
        consts = ctx.enter_context(tc.tile_pool(name="consts", bufs=1))
        xs = ctx.enter_context(tc.tile_pool(name="xs", bufs=24))
        acts = ctx.enter_context(tc.tile_pool(name="acts", bufs=1))
        exps = ctx.enter_context(tc.tile_pool(name="exps", bufs=26))
        posbp = ctx.enter_context(tc.tile_pool(name="posbp", bufs=4))
        scrp = ctx.enter_context(tc.tile_pool(name="scrp", bufs=4))
        bcsp = ctx.enter_context(tc.tile_pool(name="bcsp", bufs=2))
        osb = ctx.enter_context(tc.tile_pool(name="osb", bufs=4))
        ps_sc = ctx.enter_context(tc.tile_pool(name="ps_sc", bufs=2, space="PSUM"))
        ps1 = ctx.enter_context(tc.tile_pool(name="ps1", bufs=2, space="PSUM"))

        # ---- persistent activation tiles ----
        kt = [acts.tile([128, S], bf16, name=f"kt{m}", tag=f"kt{m}") for m in range(2)]
        qt = [acts.tile([128, S], bf16, name=f"qt{m}", tag=f"qt{m}") for m in range(2)]
        vh_sb = acts.tile([128, 16, 260], bf16, name="vh", tag="vh")
        outT2 = [
            acts.tile([128, S], bf16, name=f"outT2_{p}", tag=f"outT2_{p}")
            for p in range(2)
        ]
        # reciprocal rows live at partitions 32*(2*pair+hh); other partitions
        # stay at the memset value so the sel2 matmul contracts over zeros,
        # never garbage (0*NaN would poison the broadcast).
        rcpg = acts.tile([128, 512], bf16, name="rcpg", tag="rcpg")

        # ---- DMA issue, needed-by order, all on the sync-engine HW queue ----
        wk_sb = consts.tile([128, 8 * 256], bf16, name="wk_sb")
        nc.sync.dma_start(out=wk_sb[:], in_=wk[:])
        bk_sb = consts.tile([128, 2], f32, name="bk_sb")
        nc.sync.dma_start(out=bk_sb[:], in_=bk[:])

        xt = {}  # (input, dc, half) -> [128, 1024] bf16 tile

        def dma_x(src_, key, dc, half):
            t = xs.tile([128, S // 2], bf16, name="xt", tag="xt")
            nc.sync.dma_start(
                out=t[:],
                in_=src_[dc * 128:(dc + 1) * 128, half * 1024:(half + 1) * 1024],
            )
            xt[(key, dc, half)] = t

        for dc in range(8):
            dma_x(xkT, "k", dc, 0)
        wq_sb = consts.tile([128, 8 * 256], bf16, name="wq_sb")
        nc.sync.dma_start(out=wq_sb[:], in_=wq[:])
        bq_sb = consts.tile([128, 2], f32, name="bq_sb")
        nc.sync.dma_start(out=bq_sb[:], in_=bq[:])
        for dc in range(8):
            dma_x(xkT, "k", dc, 1)
        cm2_sb = consts.tile([128, 2, 128], bf16, name="cm2_sb")
        nc.sync.dma_start(out=cm2_sb[:], in_=cm2[:])
        sel2_sb = consts.tile([128, 256], bf16, name="sel2_sb")
        nc.sync.dma_start(out=sel2_sb[:], in_=sel2[:])
        nc.vector.memset(rcpg[:], 0.0)
        bvp_sb = consts.tile([128, 260], f32, name="bvp_sb")
        nc.sync.dma_start(out=bvp_sb[:], in_=bvp[:].to_broadcast((128, 260)))
        for half in range(2):
            for dc in range(8):
                dma_x(xqT, "q", dc, half)
        wv_sb = consts.tile([128, 8 * 260], bf16, name="wv_sb")
        nc.sync.dma_start(out=wv_sb[:], in_=wv[:])
        for half in range(2):
            for dc in range(8):
                dma_x(xvT, "v", dc, half)
        wo2_sb = consts.tile([128, 2048], bf16, name="wo2_sb")
        nc.sync.dma_start(out=wo2_sb[:], in_=wo2[:])

        # ---- projection helpers ----
        # dc is the OUTER loop so the first matmul only waits on the first
        # input tile (the PE paces with the DMA stream instead of stalling
        # for all 8 chunks).
        def kq_sc(key, wsb, bsb, dst, sc):
            ps = [ps1.tile([128, 512], f32, name="ps", tag="ps") for _ in range(2)]
            for dc in range(8):
                for m in range(2):
                    nc.tensor.matmul(
                        ps[m][:],
                        lhsT=wsb[:, dc * 256 + m * 128: dc * 256 + (m + 1) * 128],
                        rhs=xt[(key, dc, sc // 2)][
                            :, (sc % 2) * 512:(sc % 2) * 512 + 512
                        ],
                        start=(dc == 0),
                        stop=(dc == 7),
                    )
            for m in range(2):
                nc.vector.tensor_scalar_add(
                    dst[m][:, sc * 512:(sc + 1) * 512], ps[m][:], bsb[:, m:m + 1]
                )

        def v_stpair(sp):
            sts = (2 * sp, 2 * sp + 1)
            ps = [ps1.tile([128, 512], f32, name="ps", tag="ps") for _ in range(2)]
            for dc in range(8):
                for i, st in enumerate(sts):
                    nc.tensor.matmul(
                        ps[i][:, :260],
                        lhsT=xt[("v", dc, st // 8)][
                            :, (st % 8) * 128:(st % 8 + 1) * 128
                        ],
                        rhs=wv_sb[:, dc * 260:(dc + 1) * 260],
                        start=(dc == 0),
                        stop=(dc == 7),
                    )
            for i, st in enumerate(sts):
                nc.vector.tensor_add(vh_sb[:, st, :], ps[i][:, :260], bvp_sb[:])

        # ---- attention helpers ----
        def emit_scores(qc, kc, pair):
            """scores + exp (+ causal mask) for one kc block, both heads of
            the pair.  Returns (ex tile, lo) for the matching attnV."""
            j = kc - 4 * qc  # diagonal sub-block index, or negative
            lo = 128 * j if j >= 0 else 0
            pss = ps_sc.tile([128, 2, 512], f32, name="pss", tag="pss")
            for hh in range(2):
                hr = slice(64 * hh, 64 * hh + 64)
                nc.tensor.matmul(
                    pss[:, hh, lo:],
                    lhsT=kt[pair][hr, kc * 128:(kc + 1) * 128],
                    rhs=qt[pair][hr, qc * QC + lo:(qc + 1) * QC],
                    start=True,
                    stop=True,
                )
            ex = exps.tile([128, 2, 512], bf16, name="ex", tag="ex")
            nc.scalar.activation(
                ex[:, :, lo:], pss[:, :, lo:], mybir.ActivationFunctionType.Exp
            )
            if j >= 0:
                # triangular keep-mask on the 128-wide diagonal strip
                nc.vector.tensor_mul(
                    ex[:, :, lo:lo + 128], ex[:, :, lo:lo + 128], cm2_sb[:]
                )
            return ex, lo

        def emit_attnv(qc, kc, pair, po, ex, lo):
            last = 4 * qc + 3
            for hh in range(2):
                h = 2 * pair + hh
                nc.tensor.matmul(
                    po[hh][:, lo:],
                    lhsT=vh_sb[:, kc, h * 65:(h + 1) * 65],
                    rhs=ex[:, hh, lo:],
                    start=(kc == 0),
                    stop=(kc == last),
                )

        def pair_end(qc, pair, po):
            """free the po PSUM banks fast: the denominator reciprocal
            (exp(-ln x), both in the exp table already loaded) runs on ACT
            while the attn-out staging copies run on DVE — po[hh] frees after
            one ~0.8us op on each engine."""
            posb2 = posbp.tile([128, 512], bf16, name="posb2", tag="posb2")
            with nc.allow_low_precision(reason="rcp/attn-out staged bf16"):
                for hh in range(2):
                    r = 32 * (2 * pair + hh)
                    lg = scrp.tile([1, 512], f32, name="lg", tag="lg")
                    nc.scalar.activation(
                        lg[:], po[hh][64:65, :], mybir.ActivationFunctionType.Ln
                    )
                    nc.vector.tensor_copy(
                        posb2[64 * hh:64 * hh + 64, :], po[hh][0:64, :]
                    )
                    nc.scalar.activation(
                        rcpg[r:r + 1, :],
                        lg[:],
                        mybir.ActivationFunctionType.Exp,
                        scale=-1.0,
                    )
            return posb2

        def make_bc_norm(qc, pair, posb2):
            def emit():
                bcps = ps1.tile([128, 512], f32, name="ps", tag="ps")
                nc.tensor.matmul(
                    bcps[:],
                    lhsT=sel2_sb[:, pair * 128:(pair + 1) * 128],
                    rhs=rcpg[:],
                    start=True,
                    stop=True,
                )
                bcs2 = bcsp.tile([128, 512], bf16, name="bcs2", tag="bcs2")
                nc.vector.tensor_copy(bcs2[:], bcps[:])
                nc.gpsimd.tensor_mul(
                    outT2[pair][:, qc * QC:(qc + 1) * QC], posb2[:], bcs2[:]
                )
            return emit

        def make_oproj(qc, g):
            def emit():
                sti, ns = g // 2, g % 2
                st = qc * 4 + sti
                ps = ps1.tile([128, 512], f32, name="ps", tag="ps")
                for p in range(2):
                    nc.tensor.matmul(
                        ps[:],
                        lhsT=outT2[p][:, st * 128:(st + 1) * 128],
                        rhs=wo2_sb[:, p * 1024 + ns * 512: p * 1024 + ns * 512 + 512],
                        start=(p == 0),
                        stop=(p == 1),
                    )
                ot = osb.tile([128, 512], f32, name="ot", tag="ot")
                nc.vector.tensor_copy(ot[:], ps[:])
                nc.sync.dma_start(
                    out=outp[st * 128:(st + 1) * 128, ns * 512:(ns + 1) * 512],
                    in_=ot[:],
                )
            return emit

        # ---- projections, with qc=0 and qc=1 scores woven in ----
        pre = {}  # (qc, pair, kc) -> (ex, lo)
        for sc in range(4):
            kq_sc("k", wk_sb, bk_sb, kt, sc)
        kq_sc("q", wq_sb, bq_sb, qt, 0)
        for sc in range(1, 4):
            for pair in range(2):
                pre[(0, pair, sc - 1)] = emit_scores(0, sc - 1, pair)
            kq_sc("q", wq_sb, bq_sb, qt, sc)
        for pair in range(2):
            pre[(0, pair, 3)] = emit_scores(0, 3, pair)
        for sp in range(8):
            v_stpair(sp)
            for pair in range(2):
                pre[(1, pair, sp)] = emit_scores(1, sp, pair)

        # ---- attention main loop ----
        fillers = deque()

        def pop_filler():
            if fillers:
                fillers.popleft()()

        for qc in range(NQC):
            for pair in range(2):
                po = [
                    ps1.tile([65, 512], f32, name=f"po{hh}", tag="po")
                    for hh in range(2)
                ]
                nkc = 4 * qc + 4
                if qc <= 1:
                    for kc in range(nkc):
                        ex, lo = pre.pop((qc, pair, kc))
                        emit_attnv(qc, kc, pair, po, ex, lo)
                        if kc >= 1:
                            pop_filler()
                else:
                    meta = {}
                    for kc in range(nkc + LAG):
                        if kc < nkc:
                            meta[kc] = emit_scores(qc, kc, pair)
                        if kc >= LAG:
                            ex, lo = meta.pop(kc - LAG)
                            emit_attnv(qc, kc - LAG, pair, po, ex, lo)
                        if kc >= 1:
                            pop_filler()
                posb2 = pair_end(qc, pair, po)
                fillers.append(make_bc_norm(qc, pair, posb2))
                if pair == 1:
                    for g in range(8):
                        fillers.append(make_oproj(qc, g))
        while fillers:
            fillers.popleft()()

    if split:
        _split_multi_waits(nc)
    return nc


_NC_CACHE = None


def _get_nc():
    global _NC_CACHE
    if _NC_CACHE is None:
        _NC_CACHE = _build_nc()
    return _NC_CACHE


def _swizzle_w(wT, block):
    """wT [D, C] -> [128, 8*C] so that out[p, dc*C + j] = wT[dc*128 + p, j]."""
    dcs = wT.shape[0] // 128
    return np.ascontiguousarray(
        wT.reshape(dcs, 128, wT.shape[1]).transpose(1, 0, 2).reshape(128, -1)
    )


def _np_reference(q, k, v, mask, Wq, bq, Wk, bk, Wv, bv, Wo, bo):
    def split_heads(x):
        b, s, _ = x.shape
        return x.reshape(b, s, H, DK).transpose(0, 2, 1, 3)

    qh = split_heads(q @ Wq.T + bq)
    kh = split_heads(k @ Wk.T + bk)
    vh = split_heads(v @ Wv.T + bv)
    scores = np.einsum("bhqd,bhkd->bhqk", qh, kh) / np.sqrt(np.float32(DK))
    scores = np.where(mask, np.float32(-1e9), scores)
    scores = scores - scores.max(axis=-1, keepdims=True)
    e = np.exp(scores)
    attn = e / e.sum(axis=-1, keepdims=True)
    out = np.einsum("bhqk,bhkd->bhqd", attn, vh)
    out = out.transpose(0, 2, 1, 3).reshape(q.shape[0], -1, D)
    return (out @ Wo.T + bo).astype(np.float32)


def kernel(q, k, v, mask, Wq, bq, Wk, bk, Wv, bv, Wo, bo):
    import ml_dtypes

    bf16 = ml_dtypes.bfloat16

    q = np.asarray(q, np.float32)
    k = np.asarray(k, np.float32)
    v = np.asarray(v, np.float32)
    mask = np.asarray(mask, bool)
    Wq = np.asarray(Wq, np.float32)
    bq = np.asarray(bq, np.float32)
    Wk = np.asarray(Wk, np.float32)
    bk = np.asarray(bk, np.float32)
    Wv = np.asarray(Wv, np.float32)
    bv = np.asarray(bv, np.float32)
    Wo = np.asarray(Wo, np.float32)
    bo = np.asarray(bo, np.float32)

    causal = np.triu(np.ones((S, S), dtype=bool), k=1)
    if not np.array_equal(mask.reshape(S, S), causal):
        return _np_reference(q, k, v, mask, Wq, bq, Wk, bk, Wv, bv, Wo, bo)

    _install_ntff_hook()
    from concourse.bass_utils import run_bass_kernel_spmd

    nc = _get_nc()

    # triangular keep-mask for the 128-wide diagonal strip, doubled for the
    # two heads sharing one exp tile: keep iff kk <= qq
    kk = np.arange(128)[:, None]
    qq = np.arange(128)[None, :]
    tri = (kk <= qq).astype(bf16)
    cm2_np = np.concatenate([tri, tri], axis=1)  # [128, 256]

    # sel2[32*(2p+hh), p*128 + m] = 1 for m in the hh half: broadcast-select
    # the pair's two reciprocal rows (at partitions 0/32/64/96) onto 128
    sel2_np = np.zeros((128, 256), np.float32)
    for p in range(2):
        sel2_np[32 * (2 * p), p * 128:p * 128 + 64] = 1.0
        sel2_np[32 * (2 * p + 1), p * 128 + 64:p * 128 + 128] = 1.0
    sel2_np = sel2_np.astype(bf16)

    xT = {}
    for name, x in (("q", q), ("k", k), ("v", v)):
        xT[name] = [np.ascontiguousarray(x[b].T).astype(bf16) for b in range(B)]

    in_maps = []
    for c in range(N_CORES):
        b = c // 4
        g = c % 4
        hs = slice(g * HPC * DK, (g + 1) * HPC * DK)  # 256 rows of W, cols of Wo
        wq_c = _swizzle_w((SCALE * Wq[hs]).T.astype(bf16), 256)
        wk_c = _swizzle_w(Wk[hs].T.astype(bf16), 256)
        # V' with a zero weight column at h*65+64 (ones come via bias row)
        wvT = Wv[hs].T  # [1024, 256]
        wvp = np.zeros((D, 260), np.float32)
        for h in range(HPC):
            wvp[:, h * 65:h * 65 + 64] = wvT[:, h * 64:(h + 1) * 64]
        wv_c = _swizzle_w(wvp.astype(bf16), 260)
        # wo2: pair p columns hold (Wo[:, hs].T)[p*128:(p+1)*128, :]
        woT = np.ascontiguousarray(Wo[:, hs].T)  # [256, 1024]
        wo2_c = np.concatenate([woT[0:128], woT[128:256]], axis=1).astype(bf16)
        bq_c = np.ascontiguousarray(
            (SCALE * bq[hs]).reshape(2, 128).T.astype(np.float32)
        )
        bk_c = np.ascontiguousarray(bk[hs].reshape(2, 128).T.astype(np.float32))
        bvp_c = np.zeros((1, 260), np.float32)
        for h in range(HPC):
            bvp_c[0, h * 65:h * 65 + 64] = bv[hs][h * 64:(h + 1) * 64]
            bvp_c[0, h * 65 + 64] = 1.0
        in_maps.append(
            {
                "xqT": xT["q"][b],
                "xkT": xT["k"][b],
                "xvT": xT["v"][b],
                "wq": wq_c,
                "wk": wk_c,
                "wv": wv_c,
                "wo2": wo2_c,
                "bq": bq_c,
                "bk": bk_c,
                "bvp": bvp_c,
                "cm2": cm2_np,
                "sel2": sel2_np,
            }
        )

    trace = bool(os.environ.get("BASSMHA_TRACE"))
    res = run_bass_kernel_spmd(nc, in_maps, list(range(N_CORES)), trace=trace)
    kernel._last_exec_ns = res.exec_time_ns
    kernel._last_mean_exec_ns = res.mean_exec_time_ns

    out = np.zeros((B, S, D), np.float64)
    for c in range(N_CORES):
        out[c // 4] += res.results[c]["outp"].astype(np.float64)
    out += bo.astype(np.float64)
    return out.astype(np.float32)
